# revision 21
# baseline (speedup 1.0000x reference)
"""Trainium2 Bass kernel for AttentionConstrainedLoss.

Contract: kernel(atten_map [16,1600,2048] f32, gt_bboxes [16,64,7] f32) -> scalar f32.

Strategy (data-parallel over batch, 2 scenes per core on 8 cores):
  - per cell: variance over the 2048 feature dim (memory-bound, ~26 MB/core
    streamed in 26 chunks of <=128 rows); chunks are split between the ACT
    engine (copy/square accumulators, outs dumped to a bf16 scratch to halve
    SBUF writeback) and DVE bn_stats so both engines stream under the DMA.
  - box->grid assignment via the closed form flag[g] = odd(#covering boxes)
    ? last covering box : -1. The inside test |S-midS|<=half & |T-midT|<=half
    and the nearest-cell distance are linear in the 6-dim grid basis
    [x^2, y^2, xy, x, y, 1], so the geometry runs as tiny PE matmuls in
    cells-on-partitions layout (no mask transposes, no [128,1600] vector
    chain). -(min dist + delta) is folded into the dist coefficients so
    "nearest cell" is a plain <=0 test. Mask elementwise is batched 2 chunks
    per 2-bank PSUM round; S/T columns interleave so one pair-reduce replaces
    two compares.
  - both scenes' segment sums accumulate into ONE [128,4] PSUM
    (var_bn, sumsq, K2*sum^2, count) via global box ids; the per-box means,
    validity and the final scalar are finished on host in f64.
"""

from contextlib import ExitStack

import numpy as np

_CACHE = {}

# problem constants (hardcoded per spec)
B, G, D, M = 16, 1600, 2048, 64
NCORES = 8
BPC = B // NCORES          # batches per core = 2
ROWS = BPC * G             # 3200 rows of [D] per core
NCH = 13                   # 13 chunks of <=128 cells per scene (12*128 + 64)
NCHUNK = BPC * NCH         # 26 x-chunks per core

F2 = float(np.float64(102.4) / np.float64(40.0))   # 2.56 as python float

# chunks handled by the ACT accumulate path (rest use DVE bn_stats);
# first chunk on ACT (DVE does geometry smalls early), last two on DVE
ACT_CHUNKS = frozenset((0, 2, 3, 4, 6, 8, 10, 12, 14, 16, 18, 20, 22, 24))


def _build_program():
    import concourse.bacc as bacc
    import concourse.tile as tile
    from concourse import mybir

    f32 = mybir.dt.float32
    bf16 = mybir.dt.bfloat16
    op = mybir.AluOpType
    AF = mybir.ActivationFunctionType
    X = mybir.AxisListType

    nc = bacc.Bacc("TRN2", target_bir_lowering=False, debug=False,
                   enable_asserts=False, num_devices=NCORES)

    x_d = nc.declare_dram_parameter("x", [ROWS, D], f32, isOutput=False)
    bb_d = nc.declare_dram_parameter("bb", [2 * M, 7], f32, isOutput=False)
    out_d = nc.declare_dram_parameter("out", [128, 4], f32, isOutput=True)

    with tile.TileContext(nc) as tc, ExitStack() as ctx:
        singles = ctx.enter_context(tc.tile_pool(name="singles", bufs=1))
        xpool = ctx.enter_context(tc.tile_pool(name="x", bufs=1))
        bnpool = ctx.enter_context(tc.tile_pool(name="bn", bufs=3))
        mskpool = ctx.enter_context(tc.tile_pool(name="msk", bufs=2))
        tpps = ctx.enter_context(tc.tile_pool(name="tpps", bufs=2, space="PSUM"))
        mmps = ctx.enter_context(tc.tile_pool(name="mmps", bufs=1, space="PSUM"))
        segps = ctx.enter_context(tc.tile_pool(name="segps", bufs=1, space="PSUM"))
        finps = ctx.enter_context(tc.tile_pool(name="finps", bufs=1, space="PSUM"))

        # ---------------- constants (iotas on gpsimd, run at t=0) --------------
        g_t = singles.tile([128, NCH], f32)    # cell id g = 128*t + p
        nc.gpsimd.iota(g_t, pattern=[[128, NCH]], base=0,
                       channel_multiplier=1, allow_small_or_imprecise_dtypes=True)
        iota128 = singles.tile([128, 128], f32)
        nc.gpsimd.iota(iota128, pattern=[[1, 128]], base=0, channel_multiplier=0,
                       allow_small_or_imprecise_dtypes=True)
        # weights for "last covering box": global col j -> j+1, per chunk
        wrow = singles.tile([128, NCH, 128], f32)
        nc.gpsimd.iota(wrow, pattern=[[0, NCH], [1, 128]], base=1,
                       channel_multiplier=0, allow_small_or_imprecise_dtypes=True)
        ident = singles.tile([128, 128], f32)
        nc.gpsimd.iota(ident, pattern=[[1, 128]], base=0, channel_multiplier=-1,
                       allow_small_or_imprecise_dtypes=True)

        # grid basis per cell: [x^2, y^2, xy, x, y, 1]
        r40 = float(np.float32(1.0) / np.float32(40.0))
        basis = singles.tile([128, NCH, 6], f32)
        h_t = singles.tile([128, NCH], f32)
        nc.vector.tensor_scalar(out=h_t, in0=g_t, scalar1=r40, scalar2=None,
                                op0=op.mult)
        r_t = singles.tile([128, NCH], f32)
        nc.vector.tensor_scalar(out=r_t, in0=h_t, scalar1=8388608.0,
                                scalar2=8388608.0, op0=op.add, op1=op.subtract)
        gt_t = singles.tile([128, NCH], f32)
        nc.vector.tensor_tensor(out=gt_t, in0=r_t, in1=h_t, op=op.is_gt)
        fl_t = singles.tile([128, NCH], f32)   # row index = floor(g/40)
        nc.vector.tensor_tensor(out=fl_t, in0=r_t, in1=gt_t, op=op.subtract)
        col_t = singles.tile([128, NCH], f32)  # col index = g - 40*row
        nc.vector.tensor_scalar(out=col_t, in0=fl_t, scalar1=-40.0,
                                scalar2=None, op0=op.mult)
        nc.vector.tensor_tensor(out=col_t, in0=col_t, in1=g_t, op=op.add)
        bx = basis[:, :, 3]
        by = basis[:, :, 4]
        for src, dst in ((col_t, bx), (fl_t, by)):
            nc.vector.tensor_scalar(out=dst, in0=src, scalar1=0.5,
                                    scalar2=r40, op0=op.add, op1=op.mult)
            nc.vector.tensor_scalar(out=dst, in0=dst,
                                    scalar1=float(np.float32(102.4)),
                                    scalar2=float(np.float32(-51.2)),
                                    op0=op.mult, op1=op.add)
        nc.vector.tensor_tensor(out=basis[:, :, 0], in0=bx, in1=bx, op=op.mult)
        nc.vector.tensor_tensor(out=basis[:, :, 1], in0=by, in1=by, op=op.mult)
        nc.vector.tensor_tensor(out=basis[:, :, 2], in0=bx, in1=by, op=op.mult)
        nc.vector.memset(basis[:, :, 5], 1.0)

        nc.vector.tensor_scalar(out=ident, in0=ident, scalar1=0.0,
                                scalar2=None, op0=op.is_equal)
        ones128 = singles.tile([128, 1], f32)
        nc.vector.memset(ones128, 1.0)
        stats = singles.tile([128, NCHUNK, 1], f32)
        nc.vector.memset(stats, 0.0)
        # per-chunk rhs slot: (mean, var_pop, sumsq, K2*sum^2, 1); matmul
        # uses cols 1:5, unused cols stay 0
        vrhs = singles.tile([128, NCHUNK, 5], f32)
        nc.vector.memset(vrhs, 0.0)
        nc.vector.memset(vrhs[:, :, 4], 1.0)

        # first x chunks prefetch ahead of everything (ACT consumes chunk 0)
        xap0 = x_d.ap()
        xts_pre = []
        for c in range(2):
            xt = xpool.tile([128, D], f32, tag="xt", name="xt", bufs=16)
            nc.sync.dma_start(out=xt, in_=xap0[c * 128:(c + 1) * 128, :])
            xts_pre.append(xt)

        # ---------------- per-box coefs (boxes of both scenes on partitions) ----
        bb = singles.tile([128, 7], f32)
        nc.sync.dma_start(out=bb, in_=bb_d.ap())
        cx, cy = bb[:, 0:1], bb[:, 1:2]
        bl, bw = bb[:, 3:4], bb[:, 4:5]
        yaw = bb[:, 6:7]

        ratl = singles.tile([128, 1], f32)
        nc.vector.reciprocal(ratl, bl)
        nc.vector.tensor_scalar(out=ratl, in0=ratl, scalar1=F2, scalar2=1.0,
                                op0=op.mult, op1=op.max)
        nc.vector.tensor_scalar(out=ratl, in0=ratl, scalar1=6.0, scalar2=None,
                                op0=op.min)
        ratw = singles.tile([128, 1], f32)
        nc.vector.reciprocal(ratw, bw)
        nc.vector.tensor_scalar(out=ratw, in0=ratw, scalar1=F2, scalar2=1.0,
                                op0=op.mult, op1=op.max)
        nc.vector.tensor_scalar(out=ratw, in0=ratw, scalar1=6.0, scalar2=None,
                                op0=op.min)
        el = singles.tile([128, 1], f32)
        nc.vector.tensor_tensor(out=el, in0=bl, in1=ratl, op=op.mult)
        ew = singles.tile([128, 1], f32)
        nc.vector.tensor_tensor(out=ew, in0=bw, in1=ratw, op=op.mult)

        sin_t = singles.tile([128, 1], f32)
        cos_t = singles.tile([128, 1], f32)
        halfpi = singles.tile([128, 1], f32)
        nc.vector.memset(halfpi, float(np.pi / 2))
        nc.scalar.activation(sin_t, yaw, AF.Sin)
        absyaw = singles.tile([128, 1], f32)
        nc.scalar.activation(absyaw, yaw, AF.Abs)
        nc.scalar.activation(cos_t, absyaw, AF.Sin, bias=halfpi[:, 0:1],
                             scale=-1.0)

        sw = singles.tile([128, 1], f32)
        nc.vector.tensor_tensor(out=sw, in0=sin_t, in1=ew, op=op.mult)
        cw = singles.tile([128, 1], f32)
        nc.vector.tensor_tensor(out=cw, in0=cos_t, in1=ew, op=op.mult)
        cl = singles.tile([128, 1], f32)
        nc.vector.tensor_tensor(out=cl, in0=cos_t, in1=el, op=op.mult)
        sl = singles.tile([128, 1], f32)
        nc.vector.tensor_tensor(out=sl, in0=sin_t, in1=el, op=op.mult)

        t1 = singles.tile([128, 1], f32)
        t2 = singles.tile([128, 1], f32)
        nc.vector.tensor_tensor(out=t1, in0=cw, in1=cx, op=op.mult)
        nc.vector.tensor_tensor(out=t2, in0=sw, in1=cy, op=op.mult)
        midS = singles.tile([128, 1], f32)
        nc.vector.tensor_tensor(out=midS, in0=t1, in1=t2, op=op.add)
        nc.vector.tensor_tensor(out=t1, in0=sl, in1=cx, op=op.mult)
        nc.vector.tensor_tensor(out=t2, in0=cl, in1=cy, op=op.mult)
        midT = singles.tile([128, 1], f32)
        nc.vector.tensor_tensor(out=midT, in0=t1, in1=t2, op=op.subtract)
        half = singles.tile([128, 1], f32)
        nc.vector.tensor_tensor(out=half, in0=el, in1=ew, op=op.mult)
        nc.vector.tensor_scalar(out=half, in0=half, scalar1=0.5, scalar2=None,
                                op0=op.mult)
        hh2 = singles.tile([128, 1], f32)
        nc.vector.tensor_tensor(out=hh2, in0=half, in1=half, op=op.mult)

        # coef rows on basis: u = S'^2-half^2, T'^2-half^2, dist  (<=0 inside)
        coef = singles.tile([128, 18], f32)
        nc.vector.tensor_tensor(out=coef[:, 0:1], in0=cw, in1=cw, op=op.mult)
        nc.vector.tensor_tensor(out=coef[:, 1:2], in0=sw, in1=sw, op=op.mult)
        nc.vector.scalar_tensor_tensor(out=coef[:, 2:3], in0=cw, scalar=2.0,
                                       in1=sw, op0=op.mult, op1=op.mult)
        nc.vector.scalar_tensor_tensor(out=coef[:, 3:4], in0=cw, scalar=-2.0,
                                       in1=midS, op0=op.mult, op1=op.mult)
        nc.vector.scalar_tensor_tensor(out=coef[:, 4:5], in0=sw, scalar=-2.0,
                                       in1=midS, op0=op.mult, op1=op.mult)
        nc.vector.tensor_tensor(out=coef[:, 5:6], in0=midS, in1=midS, op=op.mult)
        nc.vector.tensor_tensor(out=coef[:, 5:6], in0=coef[:, 5:6], in1=hh2,
                                op=op.subtract)
        nc.vector.tensor_tensor(out=coef[:, 6:7], in0=sl, in1=sl, op=op.mult)
        nc.vector.tensor_tensor(out=coef[:, 7:8], in0=cl, in1=cl, op=op.mult)
        nc.vector.scalar_tensor_tensor(out=coef[:, 8:9], in0=sl, scalar=-2.0,
                                       in1=cl, op0=op.mult, op1=op.mult)
        nc.vector.scalar_tensor_tensor(out=coef[:, 9:10], in0=sl, scalar=-2.0,
                                       in1=midT, op0=op.mult, op1=op.mult)
        nc.vector.scalar_tensor_tensor(out=coef[:, 10:11], in0=cl, scalar=2.0,
                                       in1=midT, op0=op.mult, op1=op.mult)
        nc.vector.tensor_tensor(out=coef[:, 11:12], in0=midT, in1=midT,
                                op=op.mult)
        nc.vector.tensor_tensor(out=coef[:, 11:12], in0=coef[:, 11:12], in1=hh2,
                                op=op.subtract)
        nc.vector.memset(coef[:, 12:13], 1.0)
        nc.vector.memset(coef[:, 13:14], 1.0)
        nc.vector.memset(coef[:, 14:15], 0.0)
        nc.vector.tensor_scalar(out=coef[:, 15:16], in0=cx, scalar1=-2.0,
                                scalar2=None, op0=op.mult)
        nc.vector.tensor_scalar(out=coef[:, 16:17], in0=cy, scalar1=-2.0,
                                scalar2=None, op0=op.mult)
        nc.vector.tensor_tensor(out=coef[:, 17:18], in0=cx, in1=cx, op=op.mult)
        nc.vector.tensor_tensor(out=t1, in0=cy, in1=cy, op=op.mult)
        nc.vector.tensor_tensor(out=coef[:, 17:18], in0=coef[:, 17:18], in1=t1,
                                op=op.add)

        # ---------------- transposes: basis chunks + coef groups ---------------
        basisT = singles.tile([6, NCH * 128], f32)   # [6, 1664] cells free
        for t in range(NCH):
            ps = tpps.tile([128, 128], f32, tag="tp")
            nc.tensor.transpose(ps[:6, :], basis[:, t, :], ident)
            nc.scalar.copy(basisT[:, t * 128:(t + 1) * 128], ps[:6, :])
        # cols 0:256 = S/T interleaved per box, 256:384 = dist
        rhsST = singles.tile([6, 384], f32)
        stv = rhsST[:, 0:256].rearrange("p (c two) -> p c two", two=2)
        for i in range(3):
            ps = tpps.tile([128, 128], f32, tag="tp")
            nc.tensor.transpose(ps[:6, :], coef[:, 6 * i:6 * i + 6], ident)
            if i < 2:
                nc.scalar.copy(stv[:, :, i], ps[:6, :])
            else:
                nc.scalar.copy(rhsST[:, 256:384], ps[:6, :])

        # ---------------- nearest cell per box -> mind_b [128,128] -------------
        mind4 = singles.tile([128, 4], f32)
        for i in range(4):
            dp = mmps.tile([128, 2, 512], f32, tag="mm", bufs=2)
            nc.tensor.matmul(out=dp[:, 0, 0:416], lhsT=rhsST[:, 256:384],
                             rhs=basisT[:, i * 416:(i + 1) * 416],
                             start=True, stop=True)
            nc.vector.tensor_reduce(out=mind4[:, i:i + 1], in_=dp[:, 0, 0:416],
                                    axis=X.X, op=op.min)
        mind = singles.tile([128, 1], f32)
        nc.vector.tensor_reduce(out=mind, in_=mind4, axis=X.X, op=op.min)
        # fold -(mind+delta) into the dist constant coef, then re-transpose
        # the D block: the mask matmul's dist column becomes
        # dist-min(dist)-delta, so near <=> value <= 0. delta=0.01 >> f32
        # accumulation noise (~1e-4), << the gap to the second-nearest cell.
        mindd = singles.tile([128, 1], f32)
        nc.vector.tensor_scalar(out=mindd, in0=mind, scalar1=0.01,
                                scalar2=None, op0=op.add)
        nc.vector.tensor_tensor(out=coef[:, 17:18], in0=coef[:, 17:18],
                                in1=mindd, op=op.subtract)
        ps = tpps.tile([128, 128], f32, tag="tp")
        nc.tensor.transpose(ps[:6, :], coef[:, 12:18], ident)
        nc.scalar.copy(rhsST[:, 256:384], ps[:6, :])

        # ---------------- mask chunks, batched 2 per PSUM round ----------------
        mask_cp = singles.tile([128, NCH, 128], f32)
        for rnd0 in range(0, NCH, 2):
            nb = min(2, NCH - rnd0)
            mm = mmps.tile([128, 2, 512], f32, tag="mm", bufs=2)
            for m in range(nb):
                t = rnd0 + m
                nc.tensor.matmul(out=mm[:, m, 0:384],
                                 lhsT=basisT[:, t * 128:(t + 1) * 128],
                                 rhs=rhsST, start=True, stop=True)
            # u = max over the interleaved (S,T) pair; <=0 means inside
            u_t = mskpool.tile([128, 2, 128], f32, tag="u")
            pv = mm[:, :, 0:256].rearrange("p n (c two) -> p n c two", two=2)
            nc.vector.tensor_reduce(out=u_t[:, :nb, :], in_=pv[:, :nb],
                                    axis=X.X, op=op.max)
            n_t = mskpool.tile([128, 2, 128], f32, tag="n")
            nc.vector.tensor_scalar(out=n_t[:, :nb, :], in0=mm[:, :nb, 256:384],
                                    scalar1=0.0, scalar2=None, op0=op.is_le)
            nc.vector.scalar_tensor_tensor(out=mask_cp[:, rnd0:rnd0 + nb, :],
                                           in0=u_t[:, :nb, :], scalar=0.0,
                                           in1=n_t[:, :nb, :],
                                           op0=op.is_le, op1=op.max)
        wmask = singles.tile([128, NCH, 128], f32)
        nc.vector.tensor_tensor(out=wmask, in0=mask_cp, in1=wrow, op=op.mult)

        # ------------- flags, both scenes at once: [128, NCH, 2] ---------------
        cnt2 = singles.tile([128, NCH, 2], f32)
        wmx2 = singles.tile([128, NCH, 2], f32)
        for b in range(BPC):
            sl_ = slice(b * M, (b + 1) * M)
            nc.vector.tensor_reduce(out=cnt2[:, :, b:b + 1],
                                    in_=mask_cp[:, :, sl_], axis=X.X, op=op.add)
            nc.vector.tensor_reduce(out=wmx2[:, :, b:b + 1],
                                    in_=wmask[:, :, sl_], axis=X.X, op=op.max)
        # parity of integer cnt: odd = 4*(h - rne(h))^2 with h = cnt/2
        hpar = singles.tile([128, NCH, 2], f32)
        nc.vector.tensor_scalar(out=hpar, in0=cnt2, scalar1=0.5,
                                scalar2=None, op0=op.mult)
        rpar = singles.tile([128, NCH, 2], f32)
        nc.vector.tensor_scalar(out=rpar, in0=hpar, scalar1=8388608.0,
                                scalar2=8388608.0, op0=op.add,
                                op1=op.subtract)
        dpar = singles.tile([128, NCH, 2], f32)
        nc.vector.tensor_tensor(out=dpar, in0=hpar, in1=rpar, op=op.subtract)
        odd2 = singles.tile([128, NCH, 2], f32)
        nc.vector.scalar_tensor_tensor(out=odd2, in0=dpar, scalar=4.0,
                                       in1=dpar, op0=op.mult, op1=op.mult)
        flag2 = singles.tile([128, NCH, 2], f32)
        nc.vector.tensor_tensor(out=flag2, in0=odd2, in1=wmx2, op=op.mult)
        nc.vector.tensor_scalar(out=flag2, in0=flag2, scalar1=1.0,
                                scalar2=None, op0=op.subtract)

        # onehots, all chunks of a scene in one broadcast-AP is_equal
        # (flag==-1 rows produce zero rows)
        ohall = []
        for b in range(BPC):
            oha = singles.tile([128, NCH, 128], f32, tag=f"oha{b}")
            iob = iota128[:, :].unsqueeze(1).broadcast_to([128, NCH, 128])
            flb = flag2[:, :, b:b + 1].broadcast_to([128, NCH, 128])
            nc.vector.tensor_tensor(out=oha, in0=iob, in1=flb, op=op.is_equal)
            ohall.append(oha)

        # ---------------- variance stream + segment accumulation ---------------
        # activation outs are discarded; bf16 scratch halves SBUF writeback
        act_scr = singles.tile([128, D], bf16)
        seg = segps.tile([128, 4], f32)
        K2 = float(np.float32(-1.0 / (2048.0 * 2048.0)))   # var_pop consts
        K3 = float(np.float32(1.0 / 2048.0))
        xap = x_d.ap()

        for c in range(NCHUNK):
            b, t = c // NCH, c % NCH
            csz = 128 if t < NCH - 1 else G - 128 * (NCH - 1)
            r0 = b * G + t * 128
            if c < 2:
                xt = xts_pre[c]
            else:
                xt = xpool.tile([128, D], f32, tag="xt", name="xt", bufs=16)
                split = 4 if c == NCHUNK - 1 else (2 if c >= 20 else 1)
                for j in range(split):
                    w = D // split
                    nc.sync.dma_start(out=xt[:csz, j * w:(j + 1) * w],
                                      in_=xap[r0:r0 + csz, j * w:(j + 1) * w])
            if c in ACT_CHUNKS:
                nc.scalar.activation(act_scr[:csz, :], xt[:csz, :], AF.Copy,
                                     accum_out=stats[:csz, c, 0:1])
                nc.scalar.activation(act_scr[:csz, :], xt[:csz, :], AF.Square,
                                     accum_out=vrhs[:csz, c, 2:3])
                nc.vector.scalar_tensor_tensor(out=vrhs[:, c, 3:4],
                                               in0=stats[:, c, 0:1],
                                               scalar=K2, in1=stats[:, c, 0:1],
                                               op0=op.mult, op1=op.mult)
            else:
                st = bnpool.tile([128, 4, 6], f32, tag="bnst")
                for j in range(4):
                    nc.vector.bn_stats(out=st[:csz, j:j + 1, :],
                                       in_=xt[:csz, j * 512:(j + 1) * 512])
                # writes (mean, var_pop) into cols 0:2
                nc.vector.bn_aggr(out=vrhs[:csz, c, 0:2], in_=st[:csz])
            nc.tensor.matmul(out=seg, lhsT=ohall[b][:, t, :],
                             rhs=vrhs[:, c, 1:5],
                             start=(c == 0), stop=(c == NCHUNK - 1))

        # ---------------- ship raw segment sums; host finishes in f64 ----------
        segs = singles.tile([128, 4], f32)
        nc.vector.tensor_copy(segs, seg)
        nc.sync.dma_start(out=out_d.ap(), in_=segs)

    nc.compile()
    return nc


def _get_program():
    if "nc" not in _CACHE:
        _CACHE["nc"] = _build_program()
    return _CACHE["nc"]


def _in_maps(atten_map, gt_bboxes):
    atten_map = np.ascontiguousarray(atten_map, dtype=np.float32)
    gt_bboxes = np.ascontiguousarray(gt_bboxes, dtype=np.float32)
    return [
        {
            "x": atten_map[c * BPC:(c + 1) * BPC].reshape(ROWS, D),
            "bb": gt_bboxes[c * BPC:(c + 1) * BPC].reshape(2 * M, 7),
        }
        for c in range(NCORES)
    ]


K1 = float(np.float64(D) / (D - 1))
K3 = float(np.float32(1.0 / 2048.0))


def _combine(parts):
    # parts [ncores, 128, 4]: (var_bn_sum, sumsq_sum, K2sum2_sum, count)
    p = parts.astype(np.float64)
    v = (p[:, :, 0] + K3 * p[:, :, 1] + p[:, :, 2]) * K1
    cntm = p[:, :, 3]
    valid = cntm > 0
    means = np.where(valid, v / np.maximum(cntm, 1.0), 0.0)
    total_mean = means.sum()
    total_valid = valid.sum()
    return np.array(np.float32(-total_mean / max(total_valid, 1.0)))


def _run(atten_map, gt_bboxes, trace=False):
    from concourse.bass_utils import run_bass_kernel_spmd

    nc = _get_program()
    res = run_bass_kernel_spmd(nc, _in_maps(atten_map, gt_bboxes),
                               list(range(NCORES)), trace=trace)
    parts = np.stack([res.results[c]["out"] for c in range(NCORES)])
    return _combine(parts), res


def kernel(atten_map, gt_bboxes):
    out, _ = _run(atten_map, gt_bboxes, trace=False)
    return out


# revision 23
# speedup vs baseline: 1.0051x; 1.0051x over previous
"""Trainium2 Bass kernel for AttentionConstrainedLoss.

Contract: kernel(atten_map [16,1600,2048] f32, gt_bboxes [16,64,7] f32) -> scalar f32.

Strategy (data-parallel over batch, 2 scenes per core on 8 cores):
  - per cell: variance over the 2048 feature dim (memory-bound, ~26 MB/core
    streamed in 26 chunks of <=128 rows); chunks are split between the ACT
    engine (copy/square accumulators, outs dumped to a bf16 scratch to halve
    SBUF writeback) and DVE bn_stats so both engines stream under the DMA.
  - box->grid assignment via the closed form flag[g] = odd(#covering boxes)
    ? last covering box : -1. The inside test |S-midS|<=half & |T-midT|<=half
    and the nearest-cell distance are linear in the 6-dim grid basis
    [x^2, y^2, xy, x, y, 1], so the geometry runs as tiny PE matmuls in
    cells-on-partitions layout (no mask transposes, no [128,1600] vector
    chain). -(min dist + delta) is folded into the dist coefficients so
    "nearest cell" is a plain <=0 test. Mask elementwise is batched 2 chunks
    per 2-bank PSUM round; S/T columns interleave so one pair-reduce replaces
    two compares.
  - both scenes' segment sums accumulate into ONE [128,4] PSUM
    (var_bn, sumsq, K2*sum^2, count) via global box ids; the per-box means,
    validity and the final scalar are finished on host in f64.
"""

from contextlib import ExitStack

import numpy as np

_CACHE = {}

# problem constants (hardcoded per spec)
B, G, D, M = 16, 1600, 2048, 64
NCORES = 8
BPC = B // NCORES          # batches per core = 2
ROWS = BPC * G             # 3200 rows of [D] per core
NCH = 13                   # 13 chunks of <=128 cells per scene (12*128 + 64)
NCHUNK = BPC * NCH         # 26 x-chunks per core

F2 = float(np.float64(102.4) / np.float64(40.0))   # 2.56 as python float

# chunks handled by the ACT accumulate path (rest use DVE bn_stats);
# first chunk on ACT (DVE does geometry smalls early), last two on DVE
ACT_CHUNKS = frozenset((0, 2, 3, 4, 6, 8, 10, 12, 14, 16, 18, 20, 22, 24))


def _build_program():
    import concourse.bacc as bacc
    import concourse.tile as tile
    from concourse import mybir

    f32 = mybir.dt.float32
    bf16 = mybir.dt.bfloat16
    op = mybir.AluOpType
    AF = mybir.ActivationFunctionType
    X = mybir.AxisListType

    nc = bacc.Bacc("TRN2", target_bir_lowering=False, debug=False,
                   enable_asserts=False, num_devices=NCORES)

    x_d = nc.declare_dram_parameter("x", [ROWS, D], f32, isOutput=False)
    bb_d = nc.declare_dram_parameter("bb", [2 * M, 7], f32, isOutput=False)
    out_d = nc.declare_dram_parameter("out", [128, 4], f32, isOutput=True)

    with tile.TileContext(nc) as tc, ExitStack() as ctx:
        singles = ctx.enter_context(tc.tile_pool(name="singles", bufs=1))
        xpool = ctx.enter_context(tc.tile_pool(name="x", bufs=1))
        bnpool = ctx.enter_context(tc.tile_pool(name="bn", bufs=3))
        mskpool = ctx.enter_context(tc.tile_pool(name="msk", bufs=2))
        tpps = ctx.enter_context(tc.tile_pool(name="tpps", bufs=2, space="PSUM"))
        mmps = ctx.enter_context(tc.tile_pool(name="mmps", bufs=1, space="PSUM"))
        segps = ctx.enter_context(tc.tile_pool(name="segps", bufs=1, space="PSUM"))
        finps = ctx.enter_context(tc.tile_pool(name="finps", bufs=1, space="PSUM"))

        # ---------------- constants (iotas on gpsimd, run at t=0) --------------
        g_t = singles.tile([128, NCH], f32)    # cell id g = 128*t + p
        nc.gpsimd.iota(g_t, pattern=[[128, NCH]], base=0,
                       channel_multiplier=1, allow_small_or_imprecise_dtypes=True)
        iota128 = singles.tile([128, 128], f32)
        nc.gpsimd.iota(iota128, pattern=[[1, 128]], base=0, channel_multiplier=0,
                       allow_small_or_imprecise_dtypes=True)
        # weights for "last covering box": global col j -> j+1, per chunk
        wrow = singles.tile([128, NCH, 128], f32)
        nc.gpsimd.iota(wrow, pattern=[[0, NCH], [1, 128]], base=1,
                       channel_multiplier=0, allow_small_or_imprecise_dtypes=True)
        ident = singles.tile([128, 128], f32)
        nc.gpsimd.iota(ident, pattern=[[1, 128]], base=0, channel_multiplier=-1,
                       allow_small_or_imprecise_dtypes=True)

        # grid basis per cell: [x^2, y^2, xy, x, y, 1]
        r40 = float(np.float32(1.0) / np.float32(40.0))
        basis = singles.tile([128, NCH, 6], f32)
        h_t = singles.tile([128, NCH], f32)
        nc.vector.tensor_scalar(out=h_t, in0=g_t, scalar1=r40, scalar2=None,
                                op0=op.mult)
        r_t = singles.tile([128, NCH], f32)
        nc.vector.tensor_scalar(out=r_t, in0=h_t, scalar1=8388608.0,
                                scalar2=8388608.0, op0=op.add, op1=op.subtract)
        gt_t = singles.tile([128, NCH], f32)
        nc.vector.tensor_tensor(out=gt_t, in0=r_t, in1=h_t, op=op.is_gt)
        fl_t = singles.tile([128, NCH], f32)   # row index = floor(g/40)
        nc.vector.tensor_tensor(out=fl_t, in0=r_t, in1=gt_t, op=op.subtract)
        col_t = singles.tile([128, NCH], f32)  # col index = g - 40*row
        nc.vector.tensor_scalar(out=col_t, in0=fl_t, scalar1=-40.0,
                                scalar2=None, op0=op.mult)
        nc.vector.tensor_tensor(out=col_t, in0=col_t, in1=g_t, op=op.add)
        bx = basis[:, :, 3]
        by = basis[:, :, 4]
        for src, dst in ((col_t, bx), (fl_t, by)):
            nc.vector.tensor_scalar(out=dst, in0=src, scalar1=0.5,
                                    scalar2=r40, op0=op.add, op1=op.mult)
            nc.vector.tensor_scalar(out=dst, in0=dst,
                                    scalar1=float(np.float32(102.4)),
                                    scalar2=float(np.float32(-51.2)),
                                    op0=op.mult, op1=op.add)
        nc.vector.tensor_tensor(out=basis[:, :, 0], in0=bx, in1=bx, op=op.mult)
        nc.vector.tensor_tensor(out=basis[:, :, 1], in0=by, in1=by, op=op.mult)
        nc.vector.tensor_tensor(out=basis[:, :, 2], in0=bx, in1=by, op=op.mult)
        nc.vector.memset(basis[:, :, 5], 1.0)

        nc.vector.tensor_scalar(out=ident, in0=ident, scalar1=0.0,
                                scalar2=None, op0=op.is_equal)
        ones128 = singles.tile([128, 1], f32)
        nc.vector.memset(ones128, 1.0)
        stats = singles.tile([128, NCHUNK, 1], f32)
        nc.vector.memset(stats, 0.0)
        # per-chunk rhs slot: (mean, var_pop, sumsq, K2*sum^2, 1); matmul
        # uses cols 1:5, unused cols stay 0
        vrhs = singles.tile([128, NCHUNK, 5], f32)
        nc.vector.memset(vrhs, 0.0)
        nc.vector.memset(vrhs[:, :, 4], 1.0)

        # first x chunks prefetch ahead of everything (ACT consumes chunk 0)
        xap0 = x_d.ap()
        xts_pre = []
        for c in range(2):
            xt = xpool.tile([128, D], f32, tag="xt", name="xt", bufs=16)
            nc.sync.dma_start(out=xt, in_=xap0[c * 128:(c + 1) * 128, :])
            xts_pre.append(xt)

        # ---------------- per-box coefs (boxes of both scenes on partitions) ----
        bb = singles.tile([128, 7], f32)
        nc.sync.dma_start(out=bb, in_=bb_d.ap())
        cx, cy = bb[:, 0:1], bb[:, 1:2]
        bl, bw = bb[:, 3:4], bb[:, 4:5]
        yaw = bb[:, 6:7]

        ratl = singles.tile([128, 1], f32)
        nc.vector.reciprocal(ratl, bl)
        nc.vector.tensor_scalar(out=ratl, in0=ratl, scalar1=F2, scalar2=1.0,
                                op0=op.mult, op1=op.max)
        nc.vector.tensor_scalar(out=ratl, in0=ratl, scalar1=6.0, scalar2=None,
                                op0=op.min)
        ratw = singles.tile([128, 1], f32)
        nc.vector.reciprocal(ratw, bw)
        nc.vector.tensor_scalar(out=ratw, in0=ratw, scalar1=F2, scalar2=1.0,
                                op0=op.mult, op1=op.max)
        nc.vector.tensor_scalar(out=ratw, in0=ratw, scalar1=6.0, scalar2=None,
                                op0=op.min)
        el = singles.tile([128, 1], f32)
        nc.vector.tensor_tensor(out=el, in0=bl, in1=ratl, op=op.mult)
        ew = singles.tile([128, 1], f32)
        nc.vector.tensor_tensor(out=ew, in0=bw, in1=ratw, op=op.mult)

        sin_t = singles.tile([128, 1], f32)
        cos_t = singles.tile([128, 1], f32)
        halfpi = singles.tile([128, 1], f32)
        nc.vector.memset(halfpi, float(np.pi / 2))
        nc.scalar.activation(sin_t, yaw, AF.Sin)
        absyaw = singles.tile([128, 1], f32)
        nc.scalar.activation(absyaw, yaw, AF.Abs)
        nc.scalar.activation(cos_t, absyaw, AF.Sin, bias=halfpi[:, 0:1],
                             scale=-1.0)

        sw = singles.tile([128, 1], f32)
        nc.vector.tensor_tensor(out=sw, in0=sin_t, in1=ew, op=op.mult)
        cw = singles.tile([128, 1], f32)
        nc.vector.tensor_tensor(out=cw, in0=cos_t, in1=ew, op=op.mult)
        cl = singles.tile([128, 1], f32)
        nc.vector.tensor_tensor(out=cl, in0=cos_t, in1=el, op=op.mult)
        sl = singles.tile([128, 1], f32)
        nc.vector.tensor_tensor(out=sl, in0=sin_t, in1=el, op=op.mult)

        t1 = singles.tile([128, 1], f32)
        t2 = singles.tile([128, 1], f32)
        nc.vector.tensor_tensor(out=t1, in0=cw, in1=cx, op=op.mult)
        nc.vector.tensor_tensor(out=t2, in0=sw, in1=cy, op=op.mult)
        midS = singles.tile([128, 1], f32)
        nc.vector.tensor_tensor(out=midS, in0=t1, in1=t2, op=op.add)
        nc.vector.tensor_tensor(out=t1, in0=sl, in1=cx, op=op.mult)
        nc.vector.tensor_tensor(out=t2, in0=cl, in1=cy, op=op.mult)
        midT = singles.tile([128, 1], f32)
        nc.vector.tensor_tensor(out=midT, in0=t1, in1=t2, op=op.subtract)
        half = singles.tile([128, 1], f32)
        nc.vector.tensor_tensor(out=half, in0=el, in1=ew, op=op.mult)
        nc.vector.tensor_scalar(out=half, in0=half, scalar1=0.5, scalar2=None,
                                op0=op.mult)
        hh2 = singles.tile([128, 1], f32)
        nc.vector.tensor_tensor(out=hh2, in0=half, in1=half, op=op.mult)

        # coef rows on basis: u = S'^2-half^2, T'^2-half^2, dist  (<=0 inside)
        coef = singles.tile([128, 18], f32)
        nc.vector.tensor_tensor(out=coef[:, 0:1], in0=cw, in1=cw, op=op.mult)
        nc.vector.tensor_tensor(out=coef[:, 1:2], in0=sw, in1=sw, op=op.mult)
        nc.vector.scalar_tensor_tensor(out=coef[:, 2:3], in0=cw, scalar=2.0,
                                       in1=sw, op0=op.mult, op1=op.mult)
        nc.vector.scalar_tensor_tensor(out=coef[:, 3:4], in0=cw, scalar=-2.0,
                                       in1=midS, op0=op.mult, op1=op.mult)
        nc.vector.scalar_tensor_tensor(out=coef[:, 4:5], in0=sw, scalar=-2.0,
                                       in1=midS, op0=op.mult, op1=op.mult)
        nc.vector.tensor_tensor(out=coef[:, 5:6], in0=midS, in1=midS, op=op.mult)
        nc.vector.tensor_tensor(out=coef[:, 5:6], in0=coef[:, 5:6], in1=hh2,
                                op=op.subtract)
        nc.vector.tensor_tensor(out=coef[:, 6:7], in0=sl, in1=sl, op=op.mult)
        nc.vector.tensor_tensor(out=coef[:, 7:8], in0=cl, in1=cl, op=op.mult)
        nc.vector.scalar_tensor_tensor(out=coef[:, 8:9], in0=sl, scalar=-2.0,
                                       in1=cl, op0=op.mult, op1=op.mult)
        nc.vector.scalar_tensor_tensor(out=coef[:, 9:10], in0=sl, scalar=-2.0,
                                       in1=midT, op0=op.mult, op1=op.mult)
        nc.vector.scalar_tensor_tensor(out=coef[:, 10:11], in0=cl, scalar=2.0,
                                       in1=midT, op0=op.mult, op1=op.mult)
        nc.vector.tensor_tensor(out=coef[:, 11:12], in0=midT, in1=midT,
                                op=op.mult)
        nc.vector.tensor_tensor(out=coef[:, 11:12], in0=coef[:, 11:12], in1=hh2,
                                op=op.subtract)
        nc.vector.memset(coef[:, 12:13], 1.0)
        nc.vector.memset(coef[:, 13:14], 1.0)
        nc.vector.memset(coef[:, 14:15], 0.0)
        nc.vector.tensor_scalar(out=coef[:, 15:16], in0=cx, scalar1=-2.0,
                                scalar2=None, op0=op.mult)
        nc.vector.tensor_scalar(out=coef[:, 16:17], in0=cy, scalar1=-2.0,
                                scalar2=None, op0=op.mult)
        nc.vector.tensor_tensor(out=coef[:, 17:18], in0=cx, in1=cx, op=op.mult)
        nc.vector.tensor_tensor(out=t1, in0=cy, in1=cy, op=op.mult)
        nc.vector.tensor_tensor(out=coef[:, 17:18], in0=coef[:, 17:18], in1=t1,
                                op=op.add)

        # ---------------- transposes: basis chunks + coef groups ---------------
        basisT = singles.tile([6, NCH * 128], f32)   # [6, 1664] cells free
        for t in range(NCH):
            ps = tpps.tile([128, 128], f32, tag="tp")
            nc.tensor.transpose(ps[:6, :], basis[:, t, :], ident)
            nc.scalar.copy(basisT[:, t * 128:(t + 1) * 128], ps[:6, :])
        # cols 0:256 = S/T interleaved per box, 256:384 = dist
        rhsST = singles.tile([6, 384], f32)
        stv = rhsST[:, 0:256].rearrange("p (c two) -> p c two", two=2)
        for i in range(3):
            ps = tpps.tile([128, 128], f32, tag="tp")
            nc.tensor.transpose(ps[:6, :], coef[:, 6 * i:6 * i + 6], ident)
            if i < 2:
                nc.scalar.copy(stv[:, :, i], ps[:6, :])
            else:
                nc.scalar.copy(rhsST[:, 256:384], ps[:6, :])

        # ---------------- nearest cell per box -> mind_b [128,128] -------------
        mind4 = singles.tile([128, 4], f32)
        for i in range(4):
            dp = mmps.tile([128, 2, 512], f32, tag="mm", bufs=2)
            nc.tensor.matmul(out=dp[:, 0, 0:416], lhsT=rhsST[:, 256:384],
                             rhs=basisT[:, i * 416:(i + 1) * 416],
                             start=True, stop=True)
            nc.vector.tensor_reduce(out=mind4[:, i:i + 1], in_=dp[:, 0, 0:416],
                                    axis=X.X, op=op.min)
        mind = singles.tile([128, 1], f32)
        nc.vector.tensor_reduce(out=mind, in_=mind4, axis=X.X, op=op.min)
        # fold -(mind+delta) into the dist constant coef, then re-transpose
        # the D block: the mask matmul's dist column becomes
        # dist-min(dist)-delta, so near <=> value <= 0. delta=0.01 >> f32
        # accumulation noise (~1e-4), << the gap to the second-nearest cell.
        mindd = singles.tile([128, 1], f32)
        nc.vector.tensor_scalar(out=mindd, in0=mind, scalar1=0.01,
                                scalar2=None, op0=op.add)
        nc.vector.tensor_tensor(out=coef[:, 17:18], in0=coef[:, 17:18],
                                in1=mindd, op=op.subtract)
        ps = tpps.tile([128, 128], f32, tag="tp")
        nc.tensor.transpose(ps[:6, :], coef[:, 12:18], ident)
        nc.scalar.copy(rhsST[:, 256:384], ps[:6, :])

        # ---------------- mask chunks, batched 2 per PSUM round ----------------
        mask_cp = singles.tile([128, NCH, 128], f32)
        for rnd0 in range(0, NCH, 2):
            nb = min(2, NCH - rnd0)
            mm = mmps.tile([128, 2, 512], f32, tag="mm", bufs=2)
            for m in range(nb):
                t = rnd0 + m
                nc.tensor.matmul(out=mm[:, m, 0:384],
                                 lhsT=basisT[:, t * 128:(t + 1) * 128],
                                 rhs=rhsST, start=True, stop=True)
            # u = max over the interleaved (S,T) pair; <=0 means inside
            u_t = mskpool.tile([128, 2, 128], f32, tag="u")
            pv = mm[:, :, 0:256].rearrange("p n (c two) -> p n c two", two=2)
            nc.vector.tensor_reduce(out=u_t[:, :nb, :], in_=pv[:, :nb],
                                    axis=X.X, op=op.max)
            n_t = mskpool.tile([128, 2, 128], f32, tag="n")
            nc.vector.tensor_scalar(out=n_t[:, :nb, :], in0=mm[:, :nb, 256:384],
                                    scalar1=0.0, scalar2=None, op0=op.is_le)
            nc.vector.scalar_tensor_tensor(out=mask_cp[:, rnd0:rnd0 + nb, :],
                                           in0=u_t[:, :nb, :], scalar=0.0,
                                           in1=n_t[:, :nb, :],
                                           op0=op.is_le, op1=op.max)
        wmask = singles.tile([128, NCH, 128], f32)
        nc.vector.tensor_tensor(out=wmask, in0=mask_cp, in1=wrow, op=op.mult)

        # ------------- flags, both scenes at once: [128, NCH, 2] ---------------
        cnt2 = singles.tile([128, NCH, 2], f32)
        wmx2 = singles.tile([128, NCH, 2], f32)
        for b in range(BPC):
            sl_ = slice(b * M, (b + 1) * M)
            nc.vector.tensor_reduce(out=cnt2[:, :, b:b + 1],
                                    in_=mask_cp[:, :, sl_], axis=X.X, op=op.add)
            nc.vector.tensor_reduce(out=wmx2[:, :, b:b + 1],
                                    in_=wmask[:, :, sl_], axis=X.X, op=op.max)
        # parity of integer cnt: odd = 4*(h - rne(h))^2 with h = cnt/2
        hpar = singles.tile([128, NCH, 2], f32)
        nc.vector.tensor_scalar(out=hpar, in0=cnt2, scalar1=0.5,
                                scalar2=None, op0=op.mult)
        rpar = singles.tile([128, NCH, 2], f32)
        nc.vector.tensor_scalar(out=rpar, in0=hpar, scalar1=8388608.0,
                                scalar2=8388608.0, op0=op.add,
                                op1=op.subtract)
        dpar = singles.tile([128, NCH, 2], f32)
        nc.vector.tensor_tensor(out=dpar, in0=hpar, in1=rpar, op=op.subtract)
        odd2 = singles.tile([128, NCH, 2], f32)
        nc.vector.scalar_tensor_tensor(out=odd2, in0=dpar, scalar=4.0,
                                       in1=dpar, op0=op.mult, op1=op.mult)
        flag2 = singles.tile([128, NCH, 2], f32)
        nc.vector.tensor_tensor(out=flag2, in0=odd2, in1=wmx2, op=op.mult)
        nc.vector.tensor_scalar(out=flag2, in0=flag2, scalar1=1.0,
                                scalar2=None, op0=op.subtract)

        # onehots, all chunks of a scene in one broadcast-AP is_equal
        # (flag==-1 rows produce zero rows)
        ohall = []
        for b in range(BPC):
            oha = singles.tile([128, NCH, 128], f32, tag=f"oha{b}")
            iob = iota128[:, :].unsqueeze(1).broadcast_to([128, NCH, 128])
            flb = flag2[:, :, b:b + 1].broadcast_to([128, NCH, 128])
            nc.vector.tensor_tensor(out=oha, in0=iob, in1=flb, op=op.is_equal)
            ohall.append(oha)

        # ---------------- variance stream + segment accumulation ---------------
        # activation outs are discarded; bf16 scratch halves SBUF writeback
        act_scr = singles.tile([128, D], bf16)
        seg = segps.tile([128, 4], f32)
        K2 = float(np.float32(-1.0 / (2048.0 * 2048.0)))   # var_pop consts
        K3 = float(np.float32(1.0 / 2048.0))
        xap = x_d.ap()

        for c in range(NCHUNK):
            b, t = c // NCH, c % NCH
            csz = 128 if t < NCH - 1 else G - 128 * (NCH - 1)
            r0 = b * G + t * 128
            if c < 2:
                xt = xts_pre[c]
            else:
                xt = xpool.tile([128, D], f32, tag="xt", name="xt", bufs=16)
                split = 4 if c == NCHUNK - 1 else (2 if c >= 20 else 1)
                for j in range(split):
                    w = D // split
                    nc.sync.dma_start(out=xt[:csz, j * w:(j + 1) * w],
                                      in_=xap[r0:r0 + csz, j * w:(j + 1) * w])
            if c in ACT_CHUNKS:
                nc.scalar.activation(act_scr[:csz, :], xt[:csz, :], AF.Copy,
                                     accum_out=stats[:csz, c, 0:1])
                nc.scalar.activation(act_scr[:csz, :], xt[:csz, :], AF.Square,
                                     accum_out=vrhs[:csz, c, 2:3])
                nc.vector.scalar_tensor_tensor(out=vrhs[:, c, 3:4],
                                               in0=stats[:, c, 0:1],
                                               scalar=K2, in1=stats[:, c, 0:1],
                                               op0=op.mult, op1=op.mult)
            else:
                st = bnpool.tile([128, 4, 6], f32, tag="bnst")
                for j in range(4):
                    nc.vector.bn_stats(out=st[:csz, j:j + 1, :],
                                       in_=xt[:csz, j * 512:(j + 1) * 512])
                # writes (mean, var_pop) into cols 0:2
                nc.vector.bn_aggr(out=vrhs[:csz, c, 0:2], in_=st[:csz])
            nc.tensor.matmul(out=seg, lhsT=ohall[b][:, t, :],
                             rhs=vrhs[:, c, 1:5],
                             start=(c == 0), stop=(c == NCHUNK - 1))

        # ---------------- ship raw segment sums; host finishes in f64 ----------
        segs = singles.tile([128, 4], f32)
        nc.scalar.copy(segs, seg)
        nc.scalar.dma_start(out=out_d.ap(), in_=segs)

    nc.compile()
    return nc


def _get_program():
    if "nc" not in _CACHE:
        _CACHE["nc"] = _build_program()
    return _CACHE["nc"]


def _in_maps(atten_map, gt_bboxes):
    atten_map = np.ascontiguousarray(atten_map, dtype=np.float32)
    gt_bboxes = np.ascontiguousarray(gt_bboxes, dtype=np.float32)
    return [
        {
            "x": atten_map[c * BPC:(c + 1) * BPC].reshape(ROWS, D),
            "bb": gt_bboxes[c * BPC:(c + 1) * BPC].reshape(2 * M, 7),
        }
        for c in range(NCORES)
    ]


K1 = float(np.float64(D) / (D - 1))
K3 = float(np.float32(1.0 / 2048.0))


def _combine(parts):
    # parts [ncores, 128, 4]: (var_bn_sum, sumsq_sum, K2sum2_sum, count)
    p = parts.astype(np.float64)
    v = (p[:, :, 0] + K3 * p[:, :, 1] + p[:, :, 2]) * K1
    cntm = p[:, :, 3]
    valid = cntm > 0
    means = np.where(valid, v / np.maximum(cntm, 1.0), 0.0)
    total_mean = means.sum()
    total_valid = valid.sum()
    return np.array(np.float32(-total_mean / max(total_valid, 1.0)))


def _run(atten_map, gt_bboxes, trace=False):
    from concourse.bass_utils import run_bass_kernel_spmd

    nc = _get_program()
    res = run_bass_kernel_spmd(nc, _in_maps(atten_map, gt_bboxes),
                               list(range(NCORES)), trace=trace)
    parts = np.stack([res.results[c]["out"] for c in range(NCORES)])
    return _combine(parts), res


def kernel(atten_map, gt_bboxes):
    out, _ = _run(atten_map, gt_bboxes, trace=False)
    return out
